# revision 23
# baseline (speedup 1.0000x reference)
"""Trainium2 Bass kernel for nn_Mem_Conv2d: 3x3 same-pad conv, NCHW
x[16,32,256,256] (*) crossbar-quantized weight[32,32,3,3] + bias[32].

Strategy
--------
- Data-parallel over batch: 16 images -> 8 cores x 2 images.
- Partitions p = (n in 2, h in 2, c in 32): 4 strips of 32 channels; each
  partition holds one half-image row strip in "257-layout": row r at free
  offset r*257, where offset r*257 is a shared zero pad column (left pad of
  row r == right pad of row r-1). Strip rows r=0..129 cover image rows
  128h-1 .. 128h+128 (1-row halos); out-of-image halo rows are zeroed.
- x is cast to bf16 during the input DMA (SWDGE cast); the crossbar-
  quantized weight levels k in [-7,7] are exact in bf16, and the scale
  s = wmax/7 is folded into the PSUM eviction, so the only precision loss
  is the bf16 rounding of x (~9e-3 max rel err vs fp32 reference).
- Conv as 9 shifted matmuls (taps) accumulating in PSUM; tap (ky,kx) for
  output rows [j, j+1] reads a 2D rhs AP [32, 2, 256] at base
  (j+ky)*257 + kx (row stride 257) -> one N=512 matmul per tap covers two
  output rows; the shared-pad layout keeps every tap in-bounds and correct.
- 16-way TensorE array packing: tile (g,c) with row group g = strip,
  col group c = an 8-row block of the current 32-row phase. 2304 matmuls.
- Output: ACT evicts PSUM->SBUF res ring with scale+bias (Identity);
  per phase, 16 output DMAs of [32 och, 8 contiguous rows, 256] issued on
  the SP (sync) HWDGE ring, separate from input DMA (gpsimd SWDGE ring).

Per-core HBM traffic is 16.8 MB in + 16.8 MB out; measured ~94 us/rep via
REPS-delta, at the ~358 GB/s per-core HBM roofline.
"""

import os
import numpy as np

import concourse.bacc as bacc
import concourse.mybir as mybir
from concourse.tile import TileContext
from concourse.bass_utils import run_bass_kernel_spmd

import ml_dtypes

# problem geometry (hardcoded per harness contract)
N_IMG, C, H, W = 16, 32, 256, 256
N_CORES = 8
IMG_PER_CORE = 2
SW = 257                    # strip row stride (shared pad layout)
RSTRIP = 130                # strip rows: halo + 128 + halo
XB_FREE = RSTRIP * SW + 1   # +1: trailing pad of last row
HALF = 128                  # rows per half
NWIN = 4                    # col groups per super
NGRP = 4                    # strips (row groups)

QMAX = 7.0


def _mode():
    return os.environ.get("BASSV2_MODE", "bf16")


def build_nc(mode):
    f32 = mybir.dt.float32
    bf16 = mybir.dt.bfloat16
    f32r = mybir.dt.float32r
    xdt = bf16 if mode == "bf16" else f32

    order = os.environ.get("BASSV2_ORDER", "cg")
    indma = os.environ.get("BASSV2_INDMA", "cast")
    chunk = int(os.environ.get("BASSV2_CHUNK", "16"))
    odma = os.environ.get("BASSV2_ODMA", "both")
    evict = os.environ.get("BASSV2_EVICT", "split")  # act | dve | split
    ring_n = int(os.environ.get("BASSV2_RING", "2"))  # res ring depth
    rpm = int(os.environ.get("BASSV2_RPM", "2"))  # rows per matmul
    ph_rows = 16
    sph = int(os.environ.get("BASSV2_SPH", str(max(1, ph_rows // (4 * rpm)))))
    reps = int(os.environ.get("BASS_CONV_REPS", "1"))

    nc = bacc.Bacc("TRN2", target_bir_lowering=False)
    x_d = nc.dram_tensor("x", [IMG_PER_CORE, C, H, W], f32, kind="ExternalInput")
    w_d = nc.dram_tensor("w", [128, 9 * 32], xdt, kind="ExternalInput")
    b_d = nc.dram_tensor("b", [128, 1], f32, kind="ExternalInput")
    s_d = nc.dram_tensor("s", [128, 1], f32, kind="ExternalInput")
    o_d = nc.dram_tensor("o", [IMG_PER_CORE, C, H, W], f32, kind="ExternalOutput")

    with TileContext(nc) as tc:
        with (
            tc.tile_pool(name="sb", bufs=1) as sb,
            tc.tile_pool(name="ps", bufs=2, space="PSUM") as ps,
        ):
            xb = sb.tile([128, XB_FREE + 7], xdt)
            wt = sb.tile([128, 9 * 32], xdt)
            bt = sb.tile([128, 1], f32)
            st = sb.tile([128, 1], f32)
            # res ring: [ring][strip g][super-in-phase][256*rpm] — si,q
            # contiguous per strip so flush DMAs collapse to few AP dims
            res = sb.tile([128, ring_n * sph * NGRP * 256 * rpm], f32)
            resv = res[:, :].rearrange(
                "p (r g s q) -> p r g s q", r=ring_n, g=NGRP, s=sph
            )
            if mode == "bf16" and indma == "plain":
                stage = sb.tile([128, 2 * chunk * 256], f32)

            nc.sync.dma_start(out=wt[:, :], in_=w_d[:, :])
            nc.sync.dma_start(out=bt[:, :], in_=b_d[:, :])
            nc.sync.dma_start(out=st[:, :], in_=s_d[:, :])

            # one-time zeroing: shared pad columns (r*257 for r=0..130),
            # slack tail, and the outer halo rows (image rows -1 and 256)
            nc.gpsimd.memset(xb[:, 0 : XB_FREE : SW], 0.0)
            nc.gpsimd.memset(xb[:, XB_FREE - 1 :], 0.0)
            xbv = xb[:, 0 : RSTRIP * SW].rearrange("p (r q) -> p r q", r=RSTRIP)
            xb4 = xb[:, 0 : RSTRIP * SW].rearrange(
                "(n h c) (r q) -> n h c r q", n=2, h=2, r=RSTRIP
            )
            for n in range(2):
                nc.gpsimd.memset(xb4[n, 0, :, 0, :], 0.0)        # image row -1
                nc.gpsimd.memset(xb4[n, 1, :, RSTRIP - 1, :], 0.0)  # image row 256

            # dram view: n x h x c x half-row r x col
            xr = x_d[:, :, :, :].rearrange("n c (h r) w -> n h c r w", h=2)

            def in_dma(out_ap, in_ap):
                if out_ap.dtype != in_ap.dtype:
                    nc.gpsimd.dma_start(out=out_ap, in_=in_ap)
                else:
                    nc.sync.dma_start(out=out_ap, in_=in_ap)

            for rep in range(reps):
                # interior rows: strip rows 1..128 <- own-half rows 0..127
                nch = (HALF + chunk - 1) // chunk
                for k in range(nch):
                    r0, r1 = 1 + chunk * k, min(1 + chunk * (k + 1), 1 + HALF)
                    if mode == "bf16" and indma == "plain":
                        sl = stage[
                            :, (k % 2) * chunk * 256 : ((k % 2) + 1) * chunk * 256
                        ].rearrange("p (r q) -> p r q", r=chunk)[:, 0 : r1 - r0, :]
                        sl4 = stage[
                            :, (k % 2) * chunk * 256 : ((k % 2) + 1) * chunk * 256
                        ].rearrange(
                            "(n h c) (r q) -> n h c r q", n=2, h=2, r=chunk
                        )[:, :, :, 0 : r1 - r0, :]
                        for n in range(2):
                            for h in range(2):
                                nc.sync.dma_start(
                                    out=sl4[n, h], in_=xr[n, h, :, r0 - 1 : r1 - 1, :]
                                )
                        nc.vector.tensor_copy(xbv[:, r0:r1, 1:257], sl)
                    else:
                        for n in range(2):
                            for h in range(2):
                                in_dma(
                                    xb4[n, h, :, r0:r1, 1:257],
                                    xr[n, h, :, r0 - 1 : r1 - 1, :],
                                )
                # cross-half halo rows:
                # strip (n,0) row 129 <- image row 128; strip (n,1) row 0 <- 127
                for n in range(2):
                    in_dma(xb4[n, 0, :, RSTRIP - 1, 1:257], x_d[n, :, HALF, :])
                    in_dma(xb4[n, 1, :, 0, 1:257], x_d[n, :, HALF - 1, :])

                # super = 4 col groups x rpm rows; phase = sph supers; col
                # group c covers a contiguous rpm*sph-row block of the phase,
                # so the res ring flush writes contiguous HBM rows
                nsup = HALF // (4 * rpm)
                for s in range(nsup):
                    ph, si = divmod(s, sph)
                    pts = []
                    for g in range(NGRP):
                        pt = ps.tile(
                            [128, 256 * rpm], f32, name=f"pt{g}", tag=f"pt{g}"
                        )
                        pts.append(pt)
                    for t in range(9):
                        ky, kx = divmod(t, 3)
                        gc = (
                            [(g, c) for g in range(NGRP) for c in range(NWIN)]
                            if order == "gc"
                            else [(g, c) for c in range(NWIN) for g in range(NGRP)]
                        )
                        for g, c in gc:
                            j = 4 * rpm * sph * ph + rpm * sph * c + rpm * si
                            base = (j + ky) * SW + kx
                            lhsT = wt[32 * g : 32 * g + 32, 32 * t : 32 * t + 32]
                            if rpm == 1:
                                rhs = xb[32 * g : 32 * g + 32, base : base + 256]
                            else:
                                rhs = xb[
                                    32 * g : 32 * g + 32, base : base + rpm * SW
                                ].rearrange("p (r q) -> p r q", r=rpm)[:, :, 0:256]
                            if mode == "fp32r":
                                lhsT = lhsT.bitcast(f32r)
                                rhs = rhs.bitcast(f32r)
                            nc.tensor.matmul(
                                pts[g][32 * c : 32 * c + 32, :],
                                lhsT,
                                rhs,
                                start=(t == 0),
                                stop=(t == 8),
                                tile_position=(32 * g, 32 * c),
                            )

                    ring = ph % ring_n
                    for g in range(NGRP):
                        use_dve = evict == "dve" or (evict == "split" and g % 2 == 1)
                        if use_dve:
                            nc.vector.tensor_scalar(
                                resv[:, ring, g, si, :],
                                pts[g][:, :],
                                st[:, :],
                                bt[:, :],
                                mybir.AluOpType.mult,
                                mybir.AluOpType.add,
                            )
                        else:
                            nc.scalar.activation(
                                resv[:, ring, g, si, :],
                                pts[g][:, :],
                                mybir.ActivationFunctionType.Identity,
                                bias=bt[:, :],
                                scale=st[:, :],
                            )
                    if si == sph - 1:
                        # flush phase: per (strip, col group) DMA of
                        # [32 och, rpm*sph contiguous rows, 256]
                        blk = rpm * sph
                        y0 = ph * 4 * blk
                        for g in range(NGRP):
                            n, h = g // 2, g % 2
                            ya = HALF * h + y0
                            for c in range(4):
                                eng = (
                                    nc.scalar
                                    if odma == "scalar"
                                    or (odma == "both" and (g + c) % 2 == 0)
                                    else nc.sync
                                )
                                eng.dma_start(
                                    out=o_d[
                                        n, :, ya + blk * c : ya + blk * (c + 1), :
                                    ].rearrange("o (s r) w -> o s r w", r=rpm),
                                    in_=resv[
                                        32 * c : 32 * c + 32, ring, g, :, :
                                    ].rearrange("p s (r w) -> p s r w", r=rpm),
                                )
    nc.finalize()
    return nc


_NC_CACHE = {}


def _get_nc(mode):
    key = (
        mode,
        os.environ.get("BASS_CONV_REPS", "1"),
        os.environ.get("BASSV2_ORDER", "cg"),
        os.environ.get("BASSV2_INDMA", "cast"),
        os.environ.get("BASSV2_CHUNK", "16"),
        os.environ.get("BASSV2_SPH", ""),
        os.environ.get("BASSV2_ODMA", "both"),
        os.environ.get("BASSV2_RPM", "2"),
        os.environ.get("BASSV2_EVICT", "split"),
        os.environ.get("BASSV2_RING", "2"),
    )
    if key not in _NC_CACHE:
        _NC_CACHE[key] = build_nc(mode)
    return _NC_CACHE[key]


def _host_prep(weight, bias, mode):
    W32 = np.asarray(weight, dtype=np.float32)
    wmax = np.float32(np.max(np.abs(W32))) + np.float32(1e-12)
    k = np.round((W32 / wmax) * np.float32(QMAX))  # integral, in [-7, 7]

    wvals = k  # exact small integers (exact in bf16 and fp32)
    scale = np.float32(np.float64(wmax) / QMAX)
    np_dt = ml_dtypes.bfloat16 if mode == "bf16" else np.float32

    # lhsT layout: [i, t*32 + o], t = 3*ky + kx
    lhsT = wvals.transpose(1, 2, 3, 0).reshape(C, 9 * C)  # [i,(ky,kx,o)]
    w_rep = np.tile(lhsT, (4, 1)).astype(np_dt)
    b_rep = np.tile(np.asarray(bias, dtype=np.float32)[:, None], (4, 1))
    s_rep = np.full((128, 1), scale, dtype=np.float32)
    return w_rep, b_rep, s_rep


def kernel(x, weight, bias):
    mode = _mode()
    x = np.asarray(x, dtype=np.float32)
    w_rep, b_rep, s_rep = _host_prep(weight, bias, mode)
    nc = _get_nc(mode)

    in_maps = []
    for cid in range(N_CORES):
        in_maps.append(
            {
                "x": np.ascontiguousarray(x[2 * cid : 2 * cid + 2]),
                "w": w_rep,
                "b": b_rep,
                "s": s_rep,
            }
        )
    r = run_bass_kernel_spmd(nc, in_maps, list(range(N_CORES)))
    out = np.empty((N_IMG, C, H, W), dtype=np.float32)
    for cid in range(N_CORES):
        out[2 * cid : 2 * cid + 2] = r.results[cid]["o"]
    return out
